# revision 13
# baseline (speedup 1.0000x reference)
"""Trainium2 Bass kernel for the SE-style channel-attention block.

reference:
    avg = mean(x, spatial); mx = max(x, spatial)              # [B, C]
    gate = sigmoid(w2 @ relu(w1 @ avg) + w2 @ relu(w1 @ mx))  # [B, C]
    out = gate[:, :, None, None] * x

Sharding: data-parallel over batch, 4 samples per core on 8 cores;
w1/w2 replicated. Per core each sample ([768, 3136] fp32, 9.6 MB) is
held resident in SBUF: load once, pool, gate, scale in place, store —
HBM traffic is the read-once + write-once minimum (~77 MB/core).

Engine split per [128, 3136] chunk: ACT does the sum pooling (Copy with
accum_out) and half the gate multiplies; DVE does the max pooling and
the other half; PE does the tiny gate matmuls (w1t/w2t pre-transposed
on host so both land in natural [K, M] layout).
"""

import numpy as np

_B, _C, _H, _W = 32, 768, 56, 56
_S = _H * _W          # 3136 spatial positions
_OC = 48              # gate hidden dim
_NCORES = 8
_BPC = _B // _NCORES  # 4 samples per core
_NCH = _C // 128      # 6 channel chunks of 128 partitions

_built = None


def _build():
    from contextlib import ExitStack

    import concourse.bacc as bacc
    import concourse.bass as bass
    import concourse.tile as tile
    from concourse import mybir

    f32 = mybir.dt.float32
    AX = mybir.AxisListType
    AF = mybir.ActivationFunctionType

    # same construction as bass_test_utils.run_kernel's tile path: Bacc
    # (debug=False — no BassDebugger under the axon client), then Tile,
    # then Bacc.compile().
    nc = bacc.Bacc(
        "TRN2", target_bir_lowering=False, debug=False, num_devices=_NCORES
    )
    x = nc.declare_dram_parameter("x", [_BPC, _C, _S], f32, isOutput=False)
    w1t = nc.declare_dram_parameter("w1t", [_C, _OC], f32, isOutput=False)
    w2t = nc.declare_dram_parameter("w2t", [_OC, _C], f32, isOutput=False)
    y = nc.declare_dram_parameter("y", [_BPC, _C, _S], f32, isOutput=True)

    with tile.TileContext(nc) as tc, ExitStack() as ctx:
        wpool = ctx.enter_context(tc.tile_pool(name="w", bufs=1))
        xpool = ctx.enter_context(tc.tile_pool(name="x", bufs=2 * _NCH + 1))
        spool = ctx.enter_context(tc.tile_pool(name="s", bufs=2))
        scpool = ctx.enter_context(tc.tile_pool(name="sc", bufs=1))
        hpool = ctx.enter_context(
            tc.tile_pool(name="ph", bufs=2, space=bass.MemorySpace.PSUM)
        )
        gpool = ctx.enter_context(
            tc.tile_pool(name="pg", bufs=2, space=bass.MemorySpace.PSUM)
        )

        w1s = wpool.tile([128, _NCH * _OC], f32, tag="w1s")
        w2s = wpool.tile([_OC, _C], f32, tag="w2s")
        inv_s = 1.0 / _S

        def scale_store_chunk(b, xts, gate, k):
            """Scale chunk k of sample b in place and store it from the ACT
            HWDGE ring. Odd chunks scale on ACT (trigger follows in-stream,
            no wait); even chunks scale on DVE (the ACT trigger's
            cross-engine wait is satisfied by the time it's reached)."""
            if k % 2 == 0:
                nc.vector.tensor_scalar_mul(xts[k][:], xts[k][:], gate[:, k : k + 1])
            else:
                nc.scalar.activation(
                    xts[k][:], xts[k][:], AF.Copy, scale=gate[:, k : k + 1]
                )
            nc.scalar.dma_start(y[b, k * 128 : (k + 1) * 128, :], xts[k][:])

        def load_and_pool(b, pending):
            """DMA sample b's 6 chunks in; per-chunk sum on ACT (Copy with
            accum_out into a scratch tile so DVE and ACT never touch the
            same write set) and max on DVE — one full pass per engine.
            Chunk j of the previous sample's scale+store work (`pending`)
            is interleaved after chunk j's pooling so stores flow during
            the whole load phase instead of bunching up behind it."""
            xts = []
            # vs cols: [sum_0, max_0, sum_1, max_1, ...] per chunk
            vs = spool.tile([128, 2 * _NCH], f32, tag="vs")
            for k in range(_NCH):
                t = xpool.tile([128, _S], f32, tag="xt")
                nc.sync.dma_start(t[:], x[b, k * 128 : (k + 1) * 128, :])
                if b == 0 and k == 0:
                    # weight loads ride along after the first chunk load:
                    # off the DMA critical path, done long before matmul 1
                    for j in range(_NCH):
                        nc.sync.dma_start(
                            w1s[:, j * _OC : (j + 1) * _OC],
                            w1t[j * 128 : (j + 1) * 128, :],
                        )
                    nc.sync.dma_start(w2s[:], w2t[:])
                if pending is not None:
                    scale_store_chunk(*pending, k)
                sc = scpool.tile([128, _S], f32, tag="sc")
                nc.scalar.activation(
                    sc[:], t[:], AF.Copy, accum_out=vs[:, 2 * k : 2 * k + 1]
                )
                nc.vector.reduce_max(vs[:, 2 * k + 1 : 2 * k + 2], t[:], axis=AX.X)
                xts.append(t)
            return xts, vs

        def compute_gate(vs):
            # h = w1 @ [sum, max]: accumulate over the 6 channel chunks.
            hp = hpool.tile([_OC, 2], f32, tag="hp")
            for k in range(_NCH):
                nc.tensor.matmul(
                    hp[:],
                    w1s[:, k * _OC : (k + 1) * _OC],
                    vs[:, 2 * k : 2 * k + 2],
                    start=(k == 0),
                    stop=(k == _NCH - 1),
                )
            # z = relu(w1@avg) + relu(w1@mx); the 1/S avg scale folds
            # into the relu's input scale.
            z2 = spool.tile([_OC, 2], f32, tag="z2")
            nc.scalar.activation(z2[:, 0:1], hp[:, 0:1], AF.Relu, scale=inv_s)
            nc.scalar.activation(z2[:, 1:2], hp[:, 1:2], AF.Relu)
            z = spool.tile([_OC, 1], f32, tag="z")
            nc.vector.tensor_add(z[:], z2[:, 0:1], z2[:, 1:2])

            # gate = sigmoid(w2 @ z), one [128, 1] matmul per chunk.
            gp = gpool.tile([128, _NCH], f32, tag="gp")
            for k in range(_NCH):
                nc.tensor.matmul(gp[:, k : k + 1], w2s[:, k * 128 : (k + 1) * 128], z[:])
            gate = spool.tile([128, _NCH], f32, tag="gate")
            nc.scalar.activation(gate[:], gp[:], AF.Sigmoid)
            return gate

        # Software-pipelined emission at chunk granularity: sample b's
        # load+pool stream carries sample b-1's scales and stores along
        # with it (per-engine streams execute in program order, so this
        # is what keeps both DMA directions and both vector engines
        # continuously fed).
        pending = None
        for b in range(_BPC):
            xts, vs = load_and_pool(b, pending)
            gate = compute_gate(vs)
            pending = (b, xts, gate)
        for k in range(_NCH):
            scale_store_chunk(*pending, k)

    nc.compile()
    return nc


def _get_built():
    global _built
    if _built is None:
        _built = _build()
    return _built


def _in_maps(in_put, w1, w2):
    x = np.ascontiguousarray(in_put.reshape(_B, _C, _S), dtype=np.float32)
    w1t = np.ascontiguousarray(np.asarray(w1, dtype=np.float32).T)
    w2t = np.ascontiguousarray(np.asarray(w2, dtype=np.float32).T)
    return [
        {"x": x[i * _BPC : (i + 1) * _BPC], "w1t": w1t, "w2t": w2t}
        for i in range(_NCORES)
    ]


def kernel(in_put, w1, w2):
    from concourse.bass_utils import run_bass_kernel_spmd

    nc = _get_built()
    res = run_bass_kernel_spmd(nc, _in_maps(in_put, w1, w2), list(range(_NCORES)))
    out = np.concatenate([r["y"] for r in res.results], axis=0)
    return out.reshape(_B, _C, _H, _W)


# revision 17
# speedup vs baseline: 1.0379x; 1.0379x over previous
"""Trainium2 Bass kernel for the SE-style channel-attention block.

reference:
    avg = mean(x, spatial); mx = max(x, spatial)              # [B, C]
    gate = sigmoid(w2 @ relu(w1 @ avg) + w2 @ relu(w1 @ mx))  # [B, C]
    out = gate[:, :, None, None] * x

Sharding: data-parallel over batch, 4 samples per core on 8 cores;
w1/w2 replicated. Per core each sample ([768, 3136] fp32, 9.6 MB) is
held resident in SBUF: load once, pool, gate, scale in place, store —
HBM traffic is the read-once + write-once minimum (~77 MB/core).

Engine split per [128, 3136] chunk: ACT does the sum pooling (Copy with
accum_out) and half the gate multiplies; DVE does the max pooling and
the other half; PE does the tiny gate matmuls (w1t/w2t pre-transposed
on host so both land in natural [K, M] layout).
"""

import numpy as np

_B, _C, _H, _W = 32, 768, 56, 56
_S = _H * _W          # 3136 spatial positions
_OC = 48              # gate hidden dim
_NCORES = 8
_BPC = _B // _NCORES  # 4 samples per core
_NCH = _C // 128      # 6 channel chunks of 128 partitions

_built = None


def _build():
    from contextlib import ExitStack

    import concourse.bacc as bacc
    import concourse.bass as bass
    import concourse.tile as tile
    from concourse import mybir

    f32 = mybir.dt.float32
    AX = mybir.AxisListType
    AF = mybir.ActivationFunctionType

    # same construction as bass_test_utils.run_kernel's tile path: Bacc
    # (debug=False — no BassDebugger under the axon client), then Tile,
    # then Bacc.compile().
    nc = bacc.Bacc(
        "TRN2", target_bir_lowering=False, debug=False, num_devices=_NCORES
    )
    x = nc.declare_dram_parameter("x", [_BPC, _C, _S], f32, isOutput=False)
    w1t = nc.declare_dram_parameter("w1t", [_C, _OC], f32, isOutput=False)
    w2t = nc.declare_dram_parameter("w2t", [_OC, _C], f32, isOutput=False)
    y = nc.declare_dram_parameter("y", [_BPC, _C, _S], f32, isOutput=True)

    with tile.TileContext(nc) as tc, ExitStack() as ctx:
        wpool = ctx.enter_context(tc.tile_pool(name="w", bufs=1))
        xpool = ctx.enter_context(tc.tile_pool(name="x", bufs=2 * _NCH + 1))
        spool = ctx.enter_context(tc.tile_pool(name="s", bufs=2))
        scpool = ctx.enter_context(tc.tile_pool(name="sc", bufs=1))
        hpool = ctx.enter_context(
            tc.tile_pool(name="ph", bufs=2, space=bass.MemorySpace.PSUM)
        )
        gpool = ctx.enter_context(
            tc.tile_pool(name="pg", bufs=2, space=bass.MemorySpace.PSUM)
        )

        w1s = wpool.tile([128, _NCH * _OC], f32, tag="w1s")
        w2s = wpool.tile([_OC, _C], f32, tag="w2s")
        inv_s = 1.0 / _S

        def store_chunk(b, xts, k):
            nc.scalar.dma_start(y[b, k * 128 : (k + 1) * 128, :], xts[k][:])

        def scale_store_piece(b, xts, gate, step):
            """Step `step` (0..5) of sample b's scale+store work, shaped so
            the ACT stream never stalls: ACT-scaled odd chunks trigger
            their store immediately (same stream); DVE-scaled even chunks
            are kicked off one step ahead and their ACT-issued trigger is
            deferred two steps, behind a full ACT multiply, so the
            cross-engine wait is satisfied by the time it's reached."""
            k = step
            if k % 2 == 0:  # steps 0/2/4: ACT-scale chunk k+1, store it
                nc.scalar.activation(
                    xts[k + 1][:], xts[k + 1][:], AF.Copy, scale=gate[:, k + 1 : k + 2]
                )
                store_chunk(b, xts, k + 1)
                if k >= 2:  # store the DVE-scaled chunk from two steps ago
                    store_chunk(b, xts, k - 2)
            else:  # steps 1/3/5: DVE-scale chunk k-1
                nc.vector.tensor_scalar_mul(
                    xts[k - 1][:], xts[k - 1][:], gate[:, k - 1 : k]
                )

        def finish_stores(b, xts):
            store_chunk(b, xts, 4)

        def load_and_pool(b, pending):
            """DMA sample b's 6 chunks in; per-chunk sum on ACT (Copy with
            accum_out into a scratch tile so DVE and ACT never touch the
            same write set) and max on DVE — one full pass per engine.
            Chunk j of the previous sample's scale+store work (`pending`)
            is interleaved after chunk j's pooling so stores flow during
            the whole load phase instead of bunching up behind it."""
            xts = []
            # vs cols: [sum_0, max_0, sum_1, max_1, ...] per chunk
            vs = spool.tile([128, 2 * _NCH], f32, tag="vs")
            for k in range(_NCH):
                t = xpool.tile([128, _S], f32, tag="xt")
                nc.sync.dma_start(t[:], x[b, k * 128 : (k + 1) * 128, :])
                if b == 0 and k == 0:
                    # weight loads ride along after the first chunk load:
                    # off the DMA critical path, done long before matmul 1
                    for j in range(_NCH):
                        nc.sync.dma_start(
                            w1s[:, j * _OC : (j + 1) * _OC],
                            w1t[j * 128 : (j + 1) * 128, :],
                        )
                    nc.sync.dma_start(w2s[:], w2t[:])
                if pending is not None:
                    scale_store_piece(*pending, k)
                sc = scpool.tile([128, _S], f32, tag="sc")
                nc.scalar.activation(
                    sc[:], t[:], AF.Copy, accum_out=vs[:, 2 * k : 2 * k + 1]
                )
                nc.vector.reduce_max(vs[:, 2 * k + 1 : 2 * k + 2], t[:], axis=AX.X)
                xts.append(t)
            return xts, vs

        def compute_gate(vs):
            # h = w1 @ [sum, max]: accumulate over the 6 channel chunks.
            hp = hpool.tile([_OC, 2], f32, tag="hp")
            for k in range(_NCH):
                nc.tensor.matmul(
                    hp[:],
                    w1s[:, k * _OC : (k + 1) * _OC],
                    vs[:, 2 * k : 2 * k + 2],
                    start=(k == 0),
                    stop=(k == _NCH - 1),
                )
            # z = relu(w1@avg) + relu(w1@mx); the 1/S avg scale folds
            # into the relu's input scale.
            z2 = spool.tile([_OC, 2], f32, tag="z2")
            nc.scalar.activation(z2[:, 0:1], hp[:, 0:1], AF.Relu, scale=inv_s)
            nc.scalar.activation(z2[:, 1:2], hp[:, 1:2], AF.Relu)
            z = spool.tile([_OC, 1], f32, tag="z")
            nc.vector.tensor_add(z[:], z2[:, 0:1], z2[:, 1:2])

            # gate = sigmoid(w2 @ z), one [128, 1] matmul per chunk.
            gp = gpool.tile([128, _NCH], f32, tag="gp")
            for k in range(_NCH):
                nc.tensor.matmul(gp[:, k : k + 1], w2s[:, k * 128 : (k + 1) * 128], z[:])
            gate = spool.tile([128, _NCH], f32, tag="gate")
            nc.scalar.activation(gate[:], gp[:], AF.Sigmoid)
            return gate

        # Software-pipelined emission at chunk granularity: sample b's
        # load+pool stream carries sample b-1's scales and stores along
        # with it (per-engine streams execute in program order, so this
        # is what keeps both DMA directions and both vector engines
        # continuously fed).
        pending = None
        for b in range(_BPC):
            xts, vs = load_and_pool(b, pending)
            if pending is not None:
                finish_stores(pending[0], pending[1])
            gate = compute_gate(vs)
            pending = (b, xts, gate)
        for k in range(_NCH):
            scale_store_piece(*pending, k)
        finish_stores(pending[0], pending[1])

    nc.compile()
    return nc


def _get_built():
    global _built
    if _built is None:
        _built = _build()
    return _built


def _in_maps(in_put, w1, w2):
    x = np.ascontiguousarray(in_put.reshape(_B, _C, _S), dtype=np.float32)
    w1t = np.ascontiguousarray(np.asarray(w1, dtype=np.float32).T)
    w2t = np.ascontiguousarray(np.asarray(w2, dtype=np.float32).T)
    return [
        {"x": x[i * _BPC : (i + 1) * _BPC], "w1t": w1t, "w2t": w2t}
        for i in range(_NCORES)
    ]


def kernel(in_put, w1, w2):
    from concourse.bass_utils import run_bass_kernel_spmd

    nc = _get_built()
    res = run_bass_kernel_spmd(nc, _in_maps(in_put, w1, w2), list(range(_NCORES)))
    out = np.concatenate([r["y"] for r in res.results], axis=0)
    return out.reshape(_B, _C, _H, _W)


# revision 20
# speedup vs baseline: 1.1380x; 1.0965x over previous
"""Trainium2 Bass kernel for the SE-style channel-attention block.

reference:
    avg = mean(x, spatial); mx = max(x, spatial)              # [B, C]
    gate = sigmoid(w2 @ relu(w1 @ avg) + w2 @ relu(w1 @ mx))  # [B, C]
    out = gate[:, :, None, None] * x

Sharding: data-parallel over batch, 4 samples per core on 8 cores;
w1/w2 replicated. Per core each sample ([768, 3136] fp32, 9.6 MB) is
held resident in SBUF: load once, pool, gate, scale in place, store —
HBM traffic is the read-once + write-once minimum (~77 MB/core).

Engine split per [128, 3136] chunk: ACT does the sum pooling (Copy with
accum_out) and half the gate multiplies; DVE does the max pooling and
the other half; PE does the tiny gate matmuls (w1t/w2t pre-transposed
on host so both land in natural [K, M] layout).
"""

import numpy as np

_B, _C, _H, _W = 32, 768, 56, 56
_S = _H * _W          # 3136 spatial positions
_OC = 48              # gate hidden dim
_NCORES = 8
_BPC = _B // _NCORES  # 4 samples per core
_NCH = _C // 128      # 6 channel chunks of 128 partitions

_built = None


def _build():
    from contextlib import ExitStack

    import concourse.bacc as bacc
    import concourse.bass as bass
    import concourse.tile as tile
    from concourse import mybir

    f32 = mybir.dt.float32
    AX = mybir.AxisListType
    AF = mybir.ActivationFunctionType

    # same construction as bass_test_utils.run_kernel's tile path: Bacc
    # (debug=False — no BassDebugger under the axon client), then Tile,
    # then Bacc.compile().
    nc = bacc.Bacc(
        "TRN2", target_bir_lowering=False, debug=False, num_devices=_NCORES
    )
    x = nc.declare_dram_parameter("x", [_BPC, _C, _S], f32, isOutput=False)
    w1t = nc.declare_dram_parameter("w1t", [_C, _OC], f32, isOutput=False)
    w2t = nc.declare_dram_parameter("w2t", [_OC, _C], f32, isOutput=False)
    y = nc.declare_dram_parameter("y", [_BPC, _C, _S], f32, isOutput=True)

    with tile.TileContext(nc) as tc, ExitStack() as ctx:
        wpool = ctx.enter_context(tc.tile_pool(name="w", bufs=1))
        xpool = ctx.enter_context(tc.tile_pool(name="x", bufs=2 * _NCH + 1))
        spool = ctx.enter_context(tc.tile_pool(name="s", bufs=2))
        scpool = ctx.enter_context(tc.tile_pool(name="sc", bufs=1))
        hpool = ctx.enter_context(
            tc.tile_pool(name="ph", bufs=2, space=bass.MemorySpace.PSUM)
        )
        gpool = ctx.enter_context(
            tc.tile_pool(name="pg", bufs=2, space=bass.MemorySpace.PSUM)
        )

        w1s = wpool.tile([128, _NCH * _OC], f32, tag="w1s")
        w2s = wpool.tile([_OC, _C], f32, tag="w2s")
        inv_s = 1.0 / _S

        def scale_and_store(b, xts, gate):
            # Scale resident chunks in place — odd chunks on ACT, even on
            # DVE — and store everything from the ACT HWDGE ring so store
            # dispatch never blocks the load (sync) ring. ACT-scaled chunks
            # trigger right after their scale (same stream, no wait); the
            # DVE-scaled triggers go last so their cross-engine waits are
            # already satisfied.
            for k in range(_NCH):
                if k % 2 == 0:
                    nc.vector.tensor_scalar_mul(
                        xts[k][:], xts[k][:], gate[:, k : k + 1]
                    )
                else:
                    nc.scalar.activation(
                        xts[k][:], xts[k][:], AF.Copy, scale=gate[:, k : k + 1]
                    )
                    nc.scalar.dma_start(y[b, k * 128 : (k + 1) * 128, :], xts[k][:])
            for k in range(0, _NCH, 2):
                nc.scalar.dma_start(y[b, k * 128 : (k + 1) * 128, :], xts[k][:])

        def load_and_pool(b):
            """DMA sample b's 6 chunks in; per-chunk sum on ACT (Copy with
            accum_out into a scratch tile so DVE and ACT never touch the
            same write set) and max on DVE — one full pass per engine."""
            xts = []
            # vs cols: [sum_0, max_0, sum_1, max_1, ...] per chunk
            vs = spool.tile([128, 2 * _NCH], f32, tag="vs")
            for k in range(_NCH):
                t = xpool.tile([128, _S], f32, tag="xt")
                nc.sync.dma_start(t[:], x[b, k * 128 : (k + 1) * 128, :])
                if b == 0 and k == 0:
                    # weight loads ride along after the first chunk load:
                    # off the DMA critical path, done long before matmul 1
                    for j in range(_NCH):
                        nc.sync.dma_start(
                            w1s[:, j * _OC : (j + 1) * _OC],
                            w1t[j * 128 : (j + 1) * 128, :],
                        )
                    nc.sync.dma_start(w2s[:], w2t[:])
                sc = scpool.tile([128, _S], f32, tag="sc")
                nc.scalar.activation(
                    sc[:], t[:], AF.Copy, accum_out=vs[:, 2 * k : 2 * k + 1]
                )
                nc.vector.reduce_max(vs[:, 2 * k + 1 : 2 * k + 2], t[:], axis=AX.X)
                xts.append(t)
            return xts, vs

        def compute_gate(vs):
            # h = w1 @ [sum, max]: accumulate over the 6 channel chunks.
            hp = hpool.tile([_OC, 2], f32, tag="hp")
            for k in range(_NCH):
                nc.tensor.matmul(
                    hp[:],
                    w1s[:, k * _OC : (k + 1) * _OC],
                    vs[:, 2 * k : 2 * k + 2],
                    start=(k == 0),
                    stop=(k == _NCH - 1),
                )
            # z = relu(w1@avg) + relu(w1@mx); the 1/S avg scale folds
            # into the relu's input scale.
            z2 = spool.tile([_OC, 2], f32, tag="z2")
            nc.scalar.activation(z2[:, 0:1], hp[:, 0:1], AF.Relu, scale=inv_s)
            nc.scalar.activation(z2[:, 1:2], hp[:, 1:2], AF.Relu)
            z = spool.tile([_OC, 1], f32, tag="z")
            nc.vector.tensor_add(z[:], z2[:, 0:1], z2[:, 1:2])

            # gate = sigmoid(w2 @ z), one [128, 1] matmul per chunk.
            gp = gpool.tile([128, _NCH], f32, tag="gp")
            for k in range(_NCH):
                nc.tensor.matmul(gp[:, k : k + 1], w2s[:, k * 128 : (k + 1) * 128], z[:])
            gate = spool.tile([128, _NCH], f32, tag="gate")
            nc.scalar.activation(gate[:], gp[:], AF.Sigmoid)
            return gate

        # Software-pipelined emission: pools + gate of sample b+1 go into
        # each engine's stream BEFORE the scales/stores of sample b, so the
        # next gate is never queued behind the previous sample's multiplies
        # (per-engine streams execute in program order).
        pending = None
        for b in range(_BPC):
            xts, vs = load_and_pool(b)
            gate = compute_gate(vs)
            if pending is not None:
                scale_and_store(*pending)
            pending = (b, xts, gate)
        scale_and_store(*pending)

    nc.compile()
    return nc


def _get_built():
    global _built
    if _built is None:
        _built = _build()
    return _built


def _in_maps(in_put, w1, w2):
    x = np.ascontiguousarray(in_put.reshape(_B, _C, _S), dtype=np.float32)
    w1t = np.ascontiguousarray(np.asarray(w1, dtype=np.float32).T)
    w2t = np.ascontiguousarray(np.asarray(w2, dtype=np.float32).T)
    return [
        {"x": x[i * _BPC : (i + 1) * _BPC], "w1t": w1t, "w2t": w2t}
        for i in range(_NCORES)
    ]


def kernel(in_put, w1, w2):
    from concourse.bass_utils import run_bass_kernel_spmd

    nc = _get_built()
    res = run_bass_kernel_spmd(nc, _in_maps(in_put, w1, w2), list(range(_NCORES)))
    out = np.concatenate([r["y"] for r in res.results], axis=0)
    return out.reshape(_B, _C, _H, _W)


# revision 22
# speedup vs baseline: 1.2244x; 1.0759x over previous
"""Trainium2 Bass kernel for the SE-style channel-attention block.

reference:
    avg = mean(x, spatial); mx = max(x, spatial)              # [B, C]
    gate = sigmoid(w2 @ relu(w1 @ avg) + w2 @ relu(w1 @ mx))  # [B, C]
    out = gate[:, :, None, None] * x

Sharding: data-parallel over batch, 4 samples per core on 8 cores;
w1/w2 replicated. Per core each sample ([768, 3136] fp32, 9.6 MB) is
held resident in SBUF: load once, pool, gate, scale in place, store —
HBM traffic is the read-once + write-once minimum (~77 MB/core).

Engine split per [128, 3136] chunk: ACT does the sum pooling (Copy with
accum_out) and half the gate multiplies; DVE does the max pooling and
the other half; PE does the tiny gate matmuls (w1t/w2t pre-transposed
on host so both land in natural [K, M] layout).
"""

import numpy as np

_B, _C, _H, _W = 32, 768, 56, 56
_S = _H * _W          # 3136 spatial positions
_OC = 48              # gate hidden dim
_NCORES = 8
_BPC = _B // _NCORES  # 4 samples per core
_NCH = _C // 128      # 6 channel chunks of 128 partitions

_built = None


def _build():
    from contextlib import ExitStack

    import concourse.bacc as bacc
    import concourse.bass as bass
    import concourse.tile as tile
    from concourse import mybir

    f32 = mybir.dt.float32
    AX = mybir.AxisListType
    AF = mybir.ActivationFunctionType

    # same construction as bass_test_utils.run_kernel's tile path: Bacc
    # (debug=False — no BassDebugger under the axon client), then Tile,
    # then Bacc.compile().
    nc = bacc.Bacc(
        "TRN2", target_bir_lowering=False, debug=False, num_devices=_NCORES
    )
    x = nc.declare_dram_parameter("x", [_BPC, _C, _S], f32, isOutput=False)
    w1t = nc.declare_dram_parameter("w1t", [_C, _OC], f32, isOutput=False)
    w2t = nc.declare_dram_parameter("w2t", [_OC, _C], f32, isOutput=False)
    y = nc.declare_dram_parameter("y", [_BPC, _C, _S], f32, isOutput=True)

    with tile.TileContext(nc) as tc, ExitStack() as ctx:
        wpool = ctx.enter_context(tc.tile_pool(name="w", bufs=1))
        xpool = ctx.enter_context(tc.tile_pool(name="x", bufs=2 * _NCH + 2))
        spool = ctx.enter_context(tc.tile_pool(name="s", bufs=2))
        scpool = ctx.enter_context(tc.tile_pool(name="sc", bufs=1))
        hpool = ctx.enter_context(
            tc.tile_pool(name="ph", bufs=2, space=bass.MemorySpace.PSUM)
        )
        gpool = ctx.enter_context(
            tc.tile_pool(name="pg", bufs=2, space=bass.MemorySpace.PSUM)
        )

        w1s = wpool.tile([128, _NCH * _OC], f32, tag="w1s")
        w2s = wpool.tile([_OC, _C], f32, tag="w2s")
        inv_s = 1.0 / _S

        def scale_and_store(b, xts, gate):
            # Scale resident chunks in place — odd chunks on ACT, even on
            # DVE — and store everything from the ACT HWDGE ring so store
            # dispatch never blocks the load (sync) ring. ACT-scaled chunks
            # trigger right after their scale (same stream, no wait); the
            # DVE-scaled triggers go last so their cross-engine waits are
            # already satisfied.
            for k in range(_NCH):
                if k % 2 == 0:
                    nc.vector.tensor_scalar_mul(
                        xts[k][:], xts[k][:], gate[:, k : k + 1]
                    )
                else:
                    nc.scalar.activation(
                        xts[k][:], xts[k][:], AF.Copy, scale=gate[:, k : k + 1]
                    )
                    nc.scalar.dma_start(y[b, k * 128 : (k + 1) * 128, :], xts[k][:])
            # DVE-scaled chunks store via the sync ring instead: by the time
            # the sync sequencer reaches these (behind the next sample's
            # loads) the DVE scales are long done, and it spreads store
            # dispatch over both HWDGE rings.
            for k in range(0, _NCH, 2):
                nc.sync.dma_start(y[b, k * 128 : (k + 1) * 128, :], xts[k][:])

        def load_and_pool(b):
            """DMA sample b's 6 chunks in; per-chunk sum on ACT (Copy with
            accum_out into a scratch tile so DVE and ACT never touch the
            same write set) and max on DVE — one full pass per engine."""
            xts = []
            # vs cols: [sum_0, max_0, sum_1, max_1, ...] per chunk
            vs = spool.tile([128, 2 * _NCH], f32, tag="vs")
            for k in range(_NCH):
                t = xpool.tile([128, _S], f32, tag="xt")
                nc.sync.dma_start(t[:], x[b, k * 128 : (k + 1) * 128, :])
                if b == 0 and k == 0:
                    # weight loads ride along after the first chunk load:
                    # off the DMA critical path, done long before matmul 1
                    for j in range(_NCH):
                        nc.sync.dma_start(
                            w1s[:, j * _OC : (j + 1) * _OC],
                            w1t[j * 128 : (j + 1) * 128, :],
                        )
                    nc.sync.dma_start(w2s[:], w2t[:])
                sc = scpool.tile([128, _S], f32, tag="sc")
                nc.scalar.activation(
                    sc[:], t[:], AF.Copy, accum_out=vs[:, 2 * k : 2 * k + 1]
                )
                nc.vector.reduce_max(vs[:, 2 * k + 1 : 2 * k + 2], t[:], axis=AX.X)
                xts.append(t)
            return xts, vs

        def compute_gate(vs):
            # h = w1 @ [sum, max]: accumulate over the 6 channel chunks.
            hp = hpool.tile([_OC, 2], f32, tag="hp")
            for k in range(_NCH):
                nc.tensor.matmul(
                    hp[:],
                    w1s[:, k * _OC : (k + 1) * _OC],
                    vs[:, 2 * k : 2 * k + 2],
                    start=(k == 0),
                    stop=(k == _NCH - 1),
                )
            # z = relu(w1@avg) + relu(w1@mx); the 1/S avg scale folds
            # into the relu's input scale.
            z2 = spool.tile([_OC, 2], f32, tag="z2")
            nc.scalar.activation(z2[:, 0:1], hp[:, 0:1], AF.Relu, scale=inv_s)
            nc.scalar.activation(z2[:, 1:2], hp[:, 1:2], AF.Relu)
            z = spool.tile([_OC, 1], f32, tag="z")
            nc.vector.tensor_add(z[:], z2[:, 0:1], z2[:, 1:2])

            # gate = sigmoid(w2 @ z), one [128, 1] matmul per chunk.
            gp = gpool.tile([128, _NCH], f32, tag="gp")
            for k in range(_NCH):
                nc.tensor.matmul(gp[:, k : k + 1], w2s[:, k * 128 : (k + 1) * 128], z[:])
            gate = spool.tile([128, _NCH], f32, tag="gate")
            nc.scalar.activation(gate[:], gp[:], AF.Sigmoid)
            return gate

        # Software-pipelined emission: pools + gate of sample b+1 go into
        # each engine's stream BEFORE the scales/stores of sample b, so the
        # next gate is never queued behind the previous sample's multiplies
        # (per-engine streams execute in program order).
        pending = None
        for b in range(_BPC):
            xts, vs = load_and_pool(b)
            gate = compute_gate(vs)
            if pending is not None:
                scale_and_store(*pending)
            pending = (b, xts, gate)
        scale_and_store(*pending)

    nc.compile()
    return nc


def _get_built():
    global _built
    if _built is None:
        _built = _build()
    return _built


def _in_maps(in_put, w1, w2):
    x = np.ascontiguousarray(in_put.reshape(_B, _C, _S), dtype=np.float32)
    w1t = np.ascontiguousarray(np.asarray(w1, dtype=np.float32).T)
    w2t = np.ascontiguousarray(np.asarray(w2, dtype=np.float32).T)
    return [
        {"x": x[i * _BPC : (i + 1) * _BPC], "w1t": w1t, "w2t": w2t}
        for i in range(_NCORES)
    ]


def kernel(in_put, w1, w2):
    from concourse.bass_utils import run_bass_kernel_spmd

    nc = _get_built()
    res = run_bass_kernel_spmd(nc, _in_maps(in_put, w1, w2), list(range(_NCORES)))
    out = np.concatenate([r["y"] for r in res.results], axis=0)
    return out.reshape(_B, _C, _H, _W)


# revision 24
# speedup vs baseline: 1.2496x; 1.0205x over previous
"""Trainium2 Bass kernel for the SE-style channel-attention block.

reference:
    avg = mean(x, spatial); mx = max(x, spatial)              # [B, C]
    gate = sigmoid(w2 @ relu(w1 @ avg) + w2 @ relu(w1 @ mx))  # [B, C]
    out = gate[:, :, None, None] * x

Sharding: data-parallel over batch, 4 samples per core on 8 cores;
w1/w2 replicated. Per core each sample ([768, 3136] fp32, 9.6 MB) is
held resident in SBUF: load once, pool, gate, scale in place, store —
HBM traffic is the read-once + write-once minimum (~77 MB/core).

Engine split per [128, 3136] chunk: ACT does the sum pooling (Copy with
accum_out) and half the gate multiplies; DVE does the max pooling and
the other half; PE does the tiny gate matmuls (w1t/w2t pre-transposed
on host so both land in natural [K, M] layout). Loads dispatch from the
sync HWDGE ring, stores from the ACT ring (plus sync for DVE-scaled
chunks), so the two DMA FIFOs never block each other; emission is
software-pipelined one sample deep so each sample's pooling and gate
are queued ahead of the previous sample's multiplies on every engine.
"""

import numpy as np

_B, _C, _H, _W = 32, 768, 56, 56
_S = _H * _W          # 3136 spatial positions
_OC = 48              # gate hidden dim
_NCORES = 8
_BPC = _B // _NCORES  # 4 samples per core
_NCH = _C // 128      # 6 channel chunks of 128 partitions

_built = None


def _build():
    from contextlib import ExitStack

    import concourse.bacc as bacc
    import concourse.bass as bass
    import concourse.tile as tile
    from concourse import mybir

    f32 = mybir.dt.float32
    AX = mybir.AxisListType
    AF = mybir.ActivationFunctionType

    # same construction as bass_test_utils.run_kernel's tile path: Bacc
    # (debug=False — no BassDebugger under the axon client), then Tile,
    # then Bacc.compile().
    nc = bacc.Bacc(
        "TRN2", target_bir_lowering=False, debug=False, num_devices=_NCORES
    )
    x = nc.declare_dram_parameter("x", [_BPC, _C, _S], f32, isOutput=False)
    w1t = nc.declare_dram_parameter("w1t", [_C, _OC], f32, isOutput=False)
    w2t = nc.declare_dram_parameter("w2t", [_OC, _C], f32, isOutput=False)
    y = nc.declare_dram_parameter("y", [_BPC, _C, _S], f32, isOutput=True)

    with tile.TileContext(nc) as tc, ExitStack() as ctx:
        wpool = ctx.enter_context(tc.tile_pool(name="w", bufs=1))
        xpool = ctx.enter_context(tc.tile_pool(name="x", bufs=2 * _NCH + 2))
        spool = ctx.enter_context(tc.tile_pool(name="s", bufs=2))
        scpool = ctx.enter_context(tc.tile_pool(name="sc", bufs=1))
        hpool = ctx.enter_context(
            tc.tile_pool(name="ph", bufs=2, space=bass.MemorySpace.PSUM)
        )
        gpool = ctx.enter_context(
            tc.tile_pool(name="pg", bufs=2, space=bass.MemorySpace.PSUM)
        )

        w1s = wpool.tile([128, _NCH * _OC], f32, tag="w1s")
        w2s = wpool.tile([_OC, _C], f32, tag="w2s")
        inv_s = 1.0 / _S

        def scale_and_store(b, xts, gate):
            # Scale resident chunks in place — odd chunks on ACT, even on
            # DVE — and store everything from the ACT HWDGE ring so store
            # dispatch never blocks the load (sync) ring. ACT-scaled chunks
            # trigger right after their scale (same stream, no wait); the
            # DVE-scaled triggers go last so their cross-engine waits are
            # already satisfied.
            for k in range(_NCH):
                if k % 2 == 0:
                    nc.vector.tensor_scalar_mul(
                        xts[k][:], xts[k][:], gate[:, k : k + 1]
                    )
                else:
                    nc.scalar.activation(
                        xts[k][:], xts[k][:], AF.Copy, scale=gate[:, k : k + 1]
                    )
                    nc.scalar.dma_start(y[b, k * 128 : (k + 1) * 128, :], xts[k][:])
            # DVE-scaled chunks store via the sync ring instead: by the time
            # the sync sequencer reaches these (behind the next sample's
            # loads) the DVE scales are long done, and it spreads store
            # dispatch over both HWDGE rings.
            for k in range(0, _NCH, 2):
                nc.sync.dma_start(y[b, k * 128 : (k + 1) * 128, :], xts[k][:])

        def load_and_pool(b):
            """DMA sample b's 6 chunks in; per-chunk sum on ACT (Copy with
            accum_out into a scratch tile so DVE and ACT never touch the
            same write set) and max on DVE — one full pass per engine."""
            xts = []
            # vs cols: [sum_0, max_0, sum_1, max_1, ...] per chunk
            vs = spool.tile([128, 2 * _NCH], f32, tag="vs")
            for k in range(_NCH):
                t = xpool.tile([128, _S], f32, tag="xt")
                nc.sync.dma_start(t[:], x[b, k * 128 : (k + 1) * 128, :])
                if b == 0 and k == 0:
                    # weight loads ride along after the first chunk load:
                    # off the DMA critical path, done long before matmul 1
                    for j in range(_NCH):
                        nc.sync.dma_start(
                            w1s[:, j * _OC : (j + 1) * _OC],
                            w1t[j * 128 : (j + 1) * 128, :],
                        )
                    nc.sync.dma_start(w2s[:], w2t[:])
                sc = scpool.tile([128, _S], f32, tag="sc")
                nc.scalar.activation(
                    sc[:], t[:], AF.Copy, accum_out=vs[:, 2 * k : 2 * k + 1]
                )
                nc.vector.reduce_max(vs[:, 2 * k + 1 : 2 * k + 2], t[:], axis=AX.X)
                xts.append(t)
            return xts, vs

        def compute_gate(vs):
            # h = w1 @ [sum, max]: accumulate over the 6 channel chunks.
            hp = hpool.tile([_OC, 2], f32, tag="hp")
            for k in range(_NCH):
                nc.tensor.matmul(
                    hp[:],
                    w1s[:, k * _OC : (k + 1) * _OC],
                    vs[:, 2 * k : 2 * k + 2],
                    start=(k == 0),
                    stop=(k == _NCH - 1),
                )
            # z = relu(w1@avg) + relu(w1@mx); the 1/S avg scale folds
            # into the relu's input scale.
            z2 = spool.tile([_OC, 2], f32, tag="z2")
            nc.scalar.activation(z2[:, 0:1], hp[:, 0:1], AF.Relu, scale=inv_s)
            nc.scalar.activation(z2[:, 1:2], hp[:, 1:2], AF.Relu)
            z = spool.tile([_OC, 1], f32, tag="z")
            nc.vector.tensor_add(z[:], z2[:, 0:1], z2[:, 1:2])

            # gate = sigmoid(w2 @ z), one [128, 1] matmul per chunk.
            gp = gpool.tile([128, _NCH], f32, tag="gp")
            for k in range(_NCH):
                nc.tensor.matmul(gp[:, k : k + 1], w2s[:, k * 128 : (k + 1) * 128], z[:])
            gate = spool.tile([128, _NCH], f32, tag="gate")
            nc.scalar.activation(gate[:], gp[:], AF.Sigmoid)
            return gate

        # Software-pipelined emission: pools + gate of sample b+1 go into
        # each engine's stream BEFORE the scales/stores of sample b, so the
        # next gate is never queued behind the previous sample's multiplies
        # (per-engine streams execute in program order).
        pending = None
        for b in range(_BPC):
            xts, vs = load_and_pool(b)
            gate = compute_gate(vs)
            if pending is not None:
                scale_and_store(*pending)
            pending = (b, xts, gate)
        scale_and_store(*pending)

    nc.compile()
    return nc


def _get_built():
    global _built
    if _built is None:
        _built = _build()
    return _built


def _in_maps(in_put, w1, w2):
    x = np.ascontiguousarray(
        np.asarray(in_put, dtype=np.float32).reshape(_B, _C, _S)
    )
    w1t = np.ascontiguousarray(np.asarray(w1, dtype=np.float32).T)
    w2t = np.ascontiguousarray(np.asarray(w2, dtype=np.float32).T)
    return [
        {"x": x[i * _BPC : (i + 1) * _BPC], "w1t": w1t, "w2t": w2t}
        for i in range(_NCORES)
    ]


def kernel(in_put, w1, w2):
    from concourse.bass_utils import run_bass_kernel_spmd

    nc = _get_built()
    res = run_bass_kernel_spmd(nc, _in_maps(in_put, w1, w2), list(range(_NCORES)))
    out = np.concatenate([r["y"] for r in res.results], axis=0)
    return out.reshape(_B, _C, _H, _W)
